# revision 15
# baseline (speedup 1.0000x reference)
"""nn_BackgroundLoss segment-reduce kernel for 8 Trainium2 NeuronCores.

Contract: kernel(**inputs) takes the FULL unsharded inputs (w, beta, x, y,
particle_id as numpy arrays; only beta/particle_id are used by the math) and
returns the full output (a float32 scalar), running the computation on the 8
NeuronCores via a Bass/Tile SPMD kernel.

Algorithm (exact segment max, segment-sharded):
  The loss needs seg_max[p] = max beta over hits of particle p (P=50000
  segments), the count of non-empty segments with p > 0, and the pid==0
  (noise) sum/count.  Segments are sharded across the 8 cores: core c owns
  hi-blocks [49c, 49c+49) where hi = pid >> 7, i.e. pids [6272c, 6272c+6272).
  While sharding, the host performs a pure layout permutation: each hit is
  placed at (partition = pid & 127, column = rank*49 + (hi - 49*core)) of a
  [128, Kp*49] fp16 tile initialised to -1 (rank = arrival index within the
  segment, Kp = max segment size).  Each (partition, col%49) cell then holds
  one segment spread over Kp k-blocks, so the device computes the EXACT
  per-segment max with a packed pairwise-max tree over the k-blocks (wide
  fp16 tensor_tensor ops on DVE; quarters fold as the 4 chunked DMAs land),
  and presence is simply seg_max > -0.5: empty segments and out-of-range
  pids never get a hit and stay at -1.  pid==0 hits are noise, not a
  segment; the host routes them to a dedicated [128, Kn] block at the tail
  of core 0's tile, where a masked sum/count yields the noise term.  Since
  beta >= 0, sum(pres*seg) = sum(max(seg, 0)), saving the mask multiply.

  Each core reduces its partials to S[128, 4] = per-partition
  (n_present, sum max(seg,0), noise_sum, noise_cnt) and outputs S; the host
  gathers the 8 partial tensors and combines them into the scalar
  (unsharding a sum-sharded output):
      attract = (A - B)/A with A = sum S0, B = sum S1
      y = attract + SB * sum S2 / max(sum S3, 1).
"""
import sys

if '/opt/trn_rl_repo' not in sys.path:
    sys.path.insert(0, '/opt/trn_rl_repo')

import numpy as np
from concourse import bacc, tile, mybir
from concourse.bass_utils import run_bass_kernel_spmd

F32 = mybir.dt.float32
F16 = mybir.dt.float16
Alu = mybir.AluOpType

SB = 0.1
NUM_PIDS = 50_000
N_CORES = 8
NCOL = 49          # hi-blocks per core; 49*8 = 392 >= ceil(50000/128) = 391
PAD = -1.0         # sentinel; real beta is in [0, 1)

_cache: dict = {}


def _build(Kp: int, Kn: int):
    assert Kp % 4 == 0
    H = Kp // 2
    nc = bacc.Bacc("TRN2", target_bir_lowering=False, debug=False,
                   num_devices=N_CORES)
    W_d = nc.dram_tensor("W", [128, Kp * NCOL + Kn], F16,
                         kind="ExternalInput").ap()
    y_d = nc.dram_tensor("y", [128, 4], F32, kind="ExternalOutput").ap()

    with tile.TileContext(nc) as tc:
        with (
            tc.tile_pool(name="bulk", bufs=1) as bulkp,
            tc.tile_pool(name="fin", bufs=1) as finp,
        ):
            W = bulkp.tile([128, Kp * NCOL + Kn], F16, tag="W")
            mid = H * NCOL
            qw = (H // 2) * NCOL
            # 4 quarter DMAs interleaved across the two HWDGE queues; the
            # last one carries the noise tail
            nc.scalar.dma_start(out=W[:, 0:qw], in_=W_d[:, 0:qw])
            nc.sync.dma_start(out=W[:, qw:2 * qw], in_=W_d[:, qw:2 * qw])
            nc.scalar.dma_start(out=W[:, 2 * qw:3 * qw],
                                in_=W_d[:, 2 * qw:3 * qw])
            nc.sync.dma_start(out=W[:, 3 * qw:], in_=W_d[:, 3 * qw:])

            # exact per-segment max: packed pairwise-max tree (fp16 2x DVE),
            # folding each DMA half's quarters as the halves land
            def tree(cur, k, _n=[0]):
                while k > 1:
                    if k % 2 == 1:
                        nc.vector.tensor_tensor(
                            cur[:, 0:NCOL], cur[:, 0:NCOL],
                            cur[:, (k - 1) * NCOL:k * NCOL], Alu.max)
                        k -= 1
                    h = k // 2
                    _n[0] += 1
                    nxt = bulkp.tile([128, h * NCOL],
                                     F32 if h == 1 else F16,
                                     tag=f"lvl{_n[0]}")
                    nc.vector.tensor_tensor(nxt[:], cur[:, 0:h * NCOL],
                                            cur[:, h * NCOL:k * NCOL],
                                            Alu.max)
                    cur, k = nxt, h
                return cur

            Q = H // 2
            tA = bulkp.tile([128, Q * NCOL], F16, tag="tA")
            tB = bulkp.tile([128, Q * NCOL], F16, tag="tB")
            nc.vector.tensor_tensor(tA[:], W[:, 0:Q * NCOL],
                                    W[:, Q * NCOL:mid], Alu.max)
            nc.vector.tensor_tensor(tB[:], W[:, mid:mid + Q * NCOL],
                                    W[:, mid + Q * NCOL:2 * mid], Alu.max)
            tAB = bulkp.tile([128, Q * NCOL], F16, tag="tAB")
            nc.vector.tensor_tensor(tAB[:], tA[:], tB[:], Alu.max)
            seg = tree(tAB, Q)  # [128, 49] f32

            # presence and sum(pres*seg) == sum(max(seg, 0)) since beta >= 0
            pres = finp.tile([128, NCOL], F32, tag="pres")
            sr = finp.tile([128, NCOL], F32, tag="sr")
            nc.vector.tensor_scalar(pres[:], seg[:], -0.5, None, Alu.is_gt)
            nc.vector.tensor_scalar_max(sr[:], seg[:], 0.0)

            # noise partials from the tail block, on the idle GpSimd engine
            Wn = W[:, Kp * NCOL:Kp * NCOL + Kn]
            nmask = finp.tile([128, Kn], F32, tag="nmask")
            nbeta = finp.tile([128, Kn], F32, tag="nbeta")
            nc.gpsimd.tensor_scalar(nmask[:], Wn, -0.5, None, Alu.is_gt)
            nc.gpsimd.tensor_scalar_max(nbeta[:], Wn, 0.0)

            S = finp.tile([128, 4], F32, tag="S")
            nc.vector.tensor_reduce(S[:, 0:1], pres[:], mybir.AxisListType.X,
                                    Alu.add)
            nc.vector.tensor_reduce(S[:, 1:2], sr[:], mybir.AxisListType.X,
                                    Alu.add)
            nc.vector.tensor_reduce(S[:, 2:3], nbeta[:], mybir.AxisListType.X,
                                    Alu.add)
            nc.vector.tensor_reduce(S[:, 3:4], nmask[:], mybir.AxisListType.X,
                                    Alu.add)
            nc.sync.dma_start(out=y_d[:], in_=S[:])

    nc.compile()
    return nc


def _shard(beta: np.ndarray, pid: np.ndarray):
    """Layout permutation: route each hit to its segment's owner core and
    slot it at (row=pid&127, col=rank*49 + local_hi); pid==0 hits go to the
    noise tail block of core 0.  Returns per-core [128, Kp*49+Kn] fp16
    arrays (PAD = -1 in empty slots) and the shape key (Kp, Kn)."""
    n = beta.shape[0]
    counts = np.bincount(pid, minlength=NUM_PIDS)
    n0 = int(counts[0])
    Kmax = int(counts[1:].max())
    Kp = (Kmax + 3) // 4 * 4
    Kn = max(((n0 + 127) // 128 + 1) // 2 * 2, 2)

    # rank of each hit within its segment (arrival order)
    order = np.argsort(pid, kind="stable")
    starts = np.concatenate([[0], np.cumsum(counts)[:-1]])
    rank = np.empty(n, dtype=np.int64)
    rank[order] = np.arange(n, dtype=np.int64) - starts[pid[order]]

    W = np.full((N_CORES, 128, Kp * NCOL + Kn), PAD, dtype=np.float16)
    b16 = beta.astype(np.float16)

    m = pid > 0
    hi = pid[m] >> 7
    core = hi // NCOL
    col = hi - core * NCOL
    W[core, pid[m] & 127, rank[m] * NCOL + col] = b16[m]

    if n0:
        j = np.arange(n0, dtype=np.int64)
        W[0, j % 128, Kp * NCOL + j // 128] = b16[pid == 0]
    return W, (Kp, Kn)


def _postprocess(res):
    G = np.zeros(4, dtype=np.float64)
    for c in range(N_CORES):
        G += np.asarray(res[c]["y"], dtype=np.float64).sum(axis=0)
    attract = (G[0] - G[1]) / max(G[0], 1.0)
    out = attract + SB * G[2] / max(G[3], 1.0)
    return np.asarray(np.float32(out))


def kernel(w, beta, x, y, particle_id):
    beta = np.ascontiguousarray(np.asarray(beta, dtype=np.float32))
    pid = np.ascontiguousarray(np.asarray(particle_id, dtype=np.int32))

    W, key = _shard(beta, pid)
    if key not in _cache:
        _cache[key] = _build(*key)
    nc = _cache[key]

    in_maps = [{"W": W[c]} for c in range(N_CORES)]
    res = run_bass_kernel_spmd(nc, in_maps, list(range(N_CORES))).results
    return _postprocess(res)
